# revision 1
# baseline (speedup 1.0000x reference)
"""Trainium2 Bass kernel for nn_Custom_Pooling_3D.

Math (from the reference): the 0/1 matrix T encodes a fixed 2x2 spatial
sum-pool over a [I=32, J=32, C=16] layout (basis index i*512 + j*16 + c),
producing [O=16, O=16, C=16] (index oi*256 + oj*16 + c):

    y[b, oi, oj, c] = sqrt( sum_{di,dj in {0,1}} x[b, 2oi+di, 2oj+dj, c]^2 )

So T is never needed on device; the pooling structure is hardcoded.

Sharding: data-parallel over batch. 1024 rows / 8 cores = 128 rows per
core = exactly the 128 SBUF partitions. Per core: stream the 16384-wide
free dim in tapered chunks (big first, small last to shrink the drain
tail), square on ScalarE (ACT), two strided tensor_adds on VectorE
(DVE) for the 2x2 window sum, sqrt on ACT, store. The kernel is
DMA-bound: ~10.5 MiB/core of HBM traffic at ~360 GB/s (~29 us floor);
ACT (~21 us) and DVE (~14 us) hide under it. No cross-core comms.

Overlap notes: engine sequencers dispatch serially and block inside a
wait, so loads (whose waits clear early) and stores (which wait on
compute) must not share a sequencer — early stores go to the Pool/SWDGE
sequencer, the last few back on the by-then-drained SP. Load-pool depth
xp=5 makes tail loads' slot-reuse waits fire before store triggers so
the final load is not queued behind store transfers.
"""

import os
import sys

import numpy as np

for _p in ("/opt/trn_rl_repo", "/root/.axon_site/_ro/trn_rl_repo"):
    if os.path.isdir(_p) and _p not in sys.path:
        sys.path.insert(0, _p)

import concourse.tile as tile
from concourse import bacc, mybir
from concourse.bass_utils import run_bass_kernel_spmd

N_CORES = 8
BATCH = 1024
IN_F = 16384  # 32 * 32 * 16  (i, j, c)
OUT_F = 4096  # 16 * 16 * 16  (oi, oj, c)
BSH = BATCH // N_CORES  # 128 rows per core == SBUF partition count

# Input-column widths per chunk (each a multiple of 1024 so every chunk
# holds whole oi-pairs).  Front-loaded: big chunks stream while the pipe
# is DMA-bound; small chunks at the end shrink the serial drain tail.
CHUNKS = [4096, 3072, 3072, 2048, 2048, 1024, 1024]
# xp=5 matters: with fewer slots the tail loads' slot-WAR waits fire late
# and their DMA triggers lose the engine to earlier-triggered stores.
BUFS = dict(xp=5, zp=2, tp=3, rp=3, op=4)
# Early stores dispatch from the idle Pool sequencer (SWDGE) so their
# sqrt-waits can't head-of-line-block load dispatches on SP; the last few
# go back on SP, which is drained by then, to get HWDGE latency for the
# critical final stores.
STORE_ENGS = ["gpsimd"] * 4 + ["sync"] * 3

_CACHE = {}



def _build_program(chunks=None, bufs=None, store_engs=None, sq_engs=None,
                   skew_last=False):
    # skew_last=True emits the final chunk's load+square ahead of the
    # previous chunk's adds/sqrt/store; measured identical to False in the
    # cost model (the scheduler orders by readiness, not emission), so the
    # default keeps the HW-validated emission order.
    chunks = chunks or CHUNKS
    bufs = bufs or BUFS
    assert sum(chunks) == IN_F and all(c % 1024 == 0 for c in chunks)
    if sq_engs is None:
        sq_engs = ["scalar"] * len(chunks)
        if chunks == CHUNKS:
            # ACT is oversubscribed in the tail window (squares + sqrts
            # arrive faster than it drains); squaring chunk 5 on DVE is the
            # one rebalance the cost model favors (-45 ns).  HW-validated.
            sq_engs[5] = "vector"
    if store_engs is None:
        store_engs = STORE_ENGS if chunks == CHUNKS else (
            ["gpsimd"] * (len(chunks) - 1) + ["sync"]
        )

    # Bacc (not plain Bass): its compile() runs generate_event_semaphores,
    # which legalizes to TRN2's 1-wait-per-instruction limit.
    nc = bacc.Bacc("TRN2", target_bir_lowering=False, debug=False)
    f32 = mybir.dt.float32
    AF = mybir.ActivationFunctionType
    x = nc.dram_tensor("x", [BSH, IN_F], f32, kind="ExternalInput").ap()
    y = nc.dram_tensor("y", [BSH, OUT_F], f32, kind="ExternalOutput").ap()

    n = len(chunks)
    xoffs = [sum(chunks[:k]) for k in range(n)]
    yoffs = [xo // 4 for xo in xoffs]

    with tile.TileContext(nc) as tc:
        with (
            tc.tile_pool(name="xp", bufs=bufs["xp"]) as xp,
            tc.tile_pool(name="zp", bufs=bufs["zp"]) as zp,
            tc.tile_pool(name="tp", bufs=bufs["tp"]) as tp,
            tc.tile_pool(name="rp", bufs=bufs["rp"]) as rp,
            tc.tile_pool(name="op", bufs=bufs["op"]) as op,
        ):

            def front(k):
                """load + square for chunk k; returns the squared tile."""
                cin = chunks[k]
                xt = xp.tile([BSH, cin], f32, tag="xt")
                nc.sync.dma_start(xt[:, :], x[:, xoffs[k] : xoffs[k] + cin])
                # square (single writer per tile keeps sync waits low)
                zt = zp.tile([BSH, cin], f32, tag="zt")
                if sq_engs[k] == "vector":
                    nc.vector.tensor_mul(zt[:, :], xt[:, :], xt[:, :])
                else:
                    nc.scalar.activation(zt[:, :], xt[:, :], AF.Square)
                return zt

            def back(k, zt):
                """j-add, i-add, sqrt, store for chunk k."""
                cin = chunks[k]
                ni = cin // 512
                cout = cin // 4
                # j-pair add: [i, oj(16), 2, c(16)] -> [i, oj(16), c(16)]
                z = zt[:, :].rearrange(
                    "p (i oj two c) -> p i oj two c", i=ni, oj=16, two=2, c=16
                )
                tt = tp.tile([BSH, 2 * cout], f32, tag="tt")
                t4 = tt[:, :].rearrange(
                    "p (i oj c) -> p i oj c", i=ni, oj=16, c=16
                )
                nc.vector.tensor_add(t4, z[:, :, :, 0, :], z[:, :, :, 1, :])
                # i-pair add: [oi, 2, m(256)] -> [oi, m(256)]
                t3 = tt[:, :].rearrange(
                    "p (oi two m) -> p oi two m", oi=ni // 2, two=2, m=256
                )
                rt = rp.tile([BSH, cout], f32, tag="rt")
                r3 = rt[:, :].rearrange("p (oi m) -> p oi m", oi=ni // 2, m=256)
                nc.vector.tensor_add(r3, t3[:, :, 0, :], t3[:, :, 1, :])
                # sqrt to its own tile, then store (engine per store_engs)
                ot = op.tile([BSH, cout], f32, tag="ot")
                nc.scalar.activation(ot[:, :], rt[:, :], AF.Sqrt)
                getattr(nc, store_engs[k]).dma_start(
                    y[:, yoffs[k] : yoffs[k] + cout], ot[:, :]
                )

            # Emission order doubles as scheduler priority.  skew_last
            # emits the FINAL chunk's load+square before the previous
            # chunk's adds/sqrt/store, so the critical-path final square
            # outranks that chunk's non-critical sqrt on ACT (measured:
            # without this, the final square starts ~2.3 us after its
            # data lands because ACT drains queued sqrts first).
            zts = {}
            for k in range(n):
                zts[k] = front(k)
                if skew_last and k == n - 2:
                    continue
                if skew_last and k == n - 1:
                    back(n - 2, zts[n - 2])
                back(k, zts[k])
    nc.compile()
    _fuse_act_table_loads(nc, {AF.Square, AF.Sqrt})
    return nc



def _fuse_act_table_loads(nc, funcs_used):
    """bacc's insert_act_table_loads picks the first table set per function,
    which here yields two loads (square -> set 0, sqrt -> set 3) at ~2.7us
    each.  One set (sqrt_and_others) contains both; patch the first load to
    it and drop the rest.  Loads carry no sync info, so deletion is safe."""
    from concourse.hw_specs import get_activation_tables

    tabs = list(get_activation_tables(nc.m.arch).items())
    combined = next(
        (i for i, (_, fns) in enumerate(tabs) if funcs_used <= fns), None
    )
    if combined is None:
        return
    for blk in nc.m.functions[0].blocks:
        insts = blk.instructions  # live list view
        loads = [i for i in insts if type(i).__name__ == "InstLoadActFuncSet"]
        if len(loads) <= 1:
            continue
        if any(i.sync_info and (i.sync_info.on_wait or i.sync_info.on_update)
               for i in loads):
            continue
        loads[0].act_func_set_id = combined
        for extra in loads[1:]:
            insts.remove(extra)


def _run(x_full, trace=False, tmpdir=None):
    """x_full: [1024, 16384] f32. Returns (y_full [1024, 4096] f32, results obj)."""
    if "nc" not in _CACHE:
        _CACHE["nc"] = _build_program()
    nc = _CACHE["nc"]
    in_maps = [
        {"x": np.ascontiguousarray(x_full[c * BSH : (c + 1) * BSH])}
        for c in range(N_CORES)
    ]
    res = run_bass_kernel_spmd(
        nc, in_maps, list(range(N_CORES)), trace=trace, tmpdir=tmpdir
    )
    y_full = np.concatenate([res.results[c]["y"] for c in range(N_CORES)], axis=0)
    return y_full, res


def kernel(input_state, T=None, **_unused):
    x = np.asarray(input_state, dtype=np.float32)
    assert x.shape == (BATCH, IN_F), x.shape
    y, _ = _run(x, trace=False)
    return y



# revision 12
# speedup vs baseline: 1.0458x; 1.0458x over previous
"""Trainium2 Bass kernel for nn_Custom_Pooling_3D.

Math (from the reference): the 0/1 matrix T encodes a fixed 2x2 spatial
sum-pool over a [I=32, J=32, C=16] layout (basis index i*512 + j*16 + c),
producing [O=16, O=16, C=16] (index oi*256 + oj*16 + c):

    y[b, oi, oj, c] = sqrt( sum_{di,dj in {0,1}} x[b, 2oi+di, 2oj+dj, c]^2 )

So T is never needed on device; the pooling structure is hardcoded.

Sharding: data-parallel over batch. 1024 rows / 8 cores = 128 rows per
core = exactly the 128 SBUF partitions. Per core: stream the 16384-wide
free dim in tapered chunk groups (big first, tiny split tail), square,
two window adds, sqrt, store. The kernel is DMA-bound: 10 MiB/core of
HBM traffic at 360 GB/s is a ~29.1 us transfer floor through the single
DMA resource; compute hides under it. No cross-core comms.

The plan below was tuned against the TimelineSim cost model (34055 ->
32564 ns): descending chunk taper, engine assignments and pool depths
swept (fresh-process evals; the BIR list scheduler is history-
sensitive) to keep the 29.1 us DMA stream gap-free.  Two post-compile
patches shave fixed latency: the three unused const-AP memsets are
dropped so the entry barrier clears ~300 ns earlier, and the first
load's wait-free DMACopy is hoisted above the barrier so its
descriptor gen starts at t~0 (first HBM transfer at ~1.3 us).
"""

import os
import sys

import numpy as np

for _p in ("/opt/trn_rl_repo", "/root/.axon_site/_ro/trn_rl_repo"):
    if os.path.isdir(_p) and _p not in sys.path:
        sys.path.insert(0, _p)

import concourse.tile as tile
from concourse import bacc, mybir
from concourse.bass_utils import run_bass_kernel_spmd

N_CORES = 8
BATCH = 1024
IN_F = 16384  # 32 * 32 * 16  (i, j, c)
OUT_F = 4096  # 16 * 16 * 16  (oi, oj, c)
BSH = BATCH // N_CORES  # 128 rows per core == SBUF partition count

# Each group is a tuple of input-column widths loaded separately but
# merged at the i-add into one store.  Single-chunk groups need a width
# that is a multiple of 1024 (whole oi pairs); (512, 512) groups pair two
# single-i rows.  Front-loaded taper: big chunks stream while the pipe
# is DMA-bound; the split 512 tail shrinks the serial drain chain.
GROUPS = [(4096,), (3072,), (3072,), (2048,), (2048,), (1024,), (1024,)]
# Engine per square, one entry per flat sub-chunk ("scalar"=ACT Square,
# "vector"=DVE mul, "gpsimd"=Pool mul).
SQ_ENGS = ["scalar"] * 5 + ["vector", "scalar"]
# Engine per j-add / i-add, one entry per group.  The swept optimum pushes
# group 5's adds (and group 3's i-add) to the otherwise-idle Pool engine so
# DVE and ACT clear the tail chains in time for the last stores.
JADD_ENGS = ["vector"] * 5 + ["gpsimd", "vector"]
IADD_ENGS = ["vector"] * 3 + ["gpsimd", "vector", "gpsimd", "vector"]
# Engine issuing each group's store DMA.
STORE_ENGS = ["gpsimd", "gpsimd", "sync", "sync", "sync", "gpsimd", "sync"]
BUFS = dict(xp=5, zp=3, tp=3, rp=3, op=6)
OSPLIT = [1] * 7

_CACHE = {}


def _build_program(groups=None, sq_engs=None, jadd_engs=None, iadd_engs=None,
                   store_engs=None, bufs=None, osplit=None):
    groups = groups or GROUPS
    bufs = bufs or BUFS
    flat = [c for g in groups for c in g]
    n_sub = len(flat)
    n_grp = len(groups)
    assert sum(flat) == IN_F
    for g in groups:
        if len(g) == 1:
            assert g[0] % 1024 == 0
        else:
            assert g == (512, 512), g
    defaulted = groups == GROUPS
    sq_engs = sq_engs or (SQ_ENGS if defaulted else ["scalar"] * n_sub)
    jadd_engs = jadd_engs or (JADD_ENGS if defaulted else ["vector"] * n_grp)
    iadd_engs = iadd_engs or (IADD_ENGS if defaulted else ["vector"] * n_grp)
    store_engs = store_engs or (
        STORE_ENGS if defaulted else ["gpsimd"] * (n_grp - 1) + ["sync"]
    )
    # pieces the group's sqrt+store are split into (shortens the tail chain)
    osplit = osplit or (OSPLIT if n_grp == len(OSPLIT) else [1] * n_grp)

    # Bacc (not plain Bass): its compile() runs generate_event_semaphores,
    # which legalizes to TRN2's 1-wait-per-instruction limit.
    nc = bacc.Bacc("TRN2", target_bir_lowering=False, debug=False)
    f32 = mybir.dt.float32
    AF = mybir.ActivationFunctionType
    x = nc.dram_tensor("x", [BSH, IN_F], f32, kind="ExternalInput").ap()
    y = nc.dram_tensor("y", [BSH, OUT_F], f32, kind="ExternalOutput").ap()

    with tile.TileContext(nc) as tc:
        with (
            tc.tile_pool(name="xp", bufs=bufs["xp"]) as xp,
            tc.tile_pool(name="zp", bufs=bufs["zp"]) as zp,
            tc.tile_pool(name="tp", bufs=bufs["tp"]) as tp,
            tc.tile_pool(name="rp", bufs=bufs["rp"]) as rp,
            tc.tile_pool(name="op", bufs=bufs["op"]) as op,
        ):
            xoff = 0
            sub = 0
            for gi, g in enumerate(groups):
                cin = sum(g)
                cout = cin // 4
                yoff = xoff // 4
                zts = []
                for c in g:
                    xt = xp.tile([BSH, c], f32, tag="xt")
                    nc.sync.dma_start(xt[:, :], x[:, xoff : xoff + c])
                    zt = zp.tile([BSH, c], f32, tag="zt")
                    eng = sq_engs[sub]
                    if eng == "scalar":
                        nc.scalar.activation(zt[:, :], xt[:, :], AF.Square)
                    else:
                        getattr(nc, eng).tensor_mul(zt[:, :], xt[:, :], xt[:, :])
                    zts.append(zt)
                    xoff += c
                    sub += 1

                jeng = getattr(nc, jadd_engs[gi])
                ieng = getattr(nc, iadd_engs[gi])
                rt = rp.tile([BSH, cout], f32, tag="rt")
                if len(g) == 1:
                    ni = cin // 512
                    # j-pair add: [i, oj(16), 2, c(16)] -> [i, oj(16), c(16)]
                    z = zts[0][:, :].rearrange(
                        "p (i oj two c) -> p i oj two c", i=ni, oj=16, two=2, c=16
                    )
                    tt = tp.tile([BSH, 2 * cout], f32, tag="tt")
                    t4 = tt[:, :].rearrange(
                        "p (i oj c) -> p i oj c", i=ni, oj=16, c=16
                    )
                    jeng.tensor_add(t4, z[:, :, :, 0, :], z[:, :, :, 1, :])
                    # i-pair add: [oi, 2, m(256)] -> [oi, m(256)]
                    t3 = tt[:, :].rearrange(
                        "p (oi two m) -> p oi two m", oi=ni // 2, two=2, m=256
                    )
                    r3 = rt[:, :].rearrange(
                        "p (oi m) -> p oi m", oi=ni // 2, m=256
                    )
                    ieng.tensor_add(r3, t3[:, :, 0, :], t3[:, :, 1, :])
                else:
                    # split pair: j-add each 512 half, merge in one i-add
                    hts = []
                    for zt in zts:
                        z = zt[:, :].rearrange(
                            "p (oj two c) -> p oj two c", oj=16, two=2, c=16
                        )
                        ht = tp.tile([BSH, cout], f32, tag="ht")
                        h3 = ht[:, :].rearrange(
                            "p (oj c) -> p oj c", oj=16, c=16
                        )
                        jeng.tensor_add(h3, z[:, :, 0, :], z[:, :, 1, :])
                        hts.append(ht)
                    ieng.tensor_add(rt[:, :], hts[0][:, :], hts[1][:, :])
                # sqrt to its own tile, then store (optionally in pieces)
                npc = osplit[gi]
                assert cout % npc == 0
                w = cout // npc
                seng = getattr(nc, store_engs[gi])
                for pi in range(npc):
                    ot = op.tile([BSH, w], f32, tag="ot")
                    nc.scalar.activation(
                        ot[:, :], rt[:, pi * w : (pi + 1) * w], AF.Sqrt
                    )
                    seng.dma_start(
                        y[:, yoff + pi * w : yoff + (pi + 1) * w], ot[:, :]
                    )
    nc.compile()
    _fuse_act_table_loads(nc, {AF.Square, AF.Sqrt})
    _drop_dead_const_memsets(nc)
    _hoist_first_load(nc)
    return nc


def _fuse_act_table_loads(nc, funcs_used):
    """bacc's insert_act_table_loads picks the first table set per function,
    which here yields two loads (square -> set 0, sqrt -> set 3) at ~2.7us
    each.  One set (sqrt_and_others) contains both; patch the first load to
    it and drop the rest.  Loads carry no sync info, so deletion is safe."""
    from concourse.hw_specs import get_activation_tables

    tabs = list(get_activation_tables(nc.m.arch).items())
    combined = next(
        (i for i, (_, fns) in enumerate(tabs) if funcs_used <= fns), None
    )
    if combined is None:
        return
    for blk in nc.m.functions[0].blocks:
        insts = blk.instructions  # live list view
        loads = [i for i in insts if type(i).__name__ == "InstLoadActFuncSet"]
        if len(loads) <= 1:
            continue
        if any(i.sync_info and (i.sync_info.on_wait or i.sync_info.on_update)
               for i in loads):
            continue
        loads[0].act_func_set_id = combined
        for extra in loads[1:]:
            insts.remove(extra)


def _drop_dead_const_memsets(nc):
    """Bass.__init__ memsets four const scalar APs (f32 0.0, f32 1.0,
    bf16 1.0, u8 127) serially on Pool before the entry barrier; only
    (f32, 0.0) — the activation bias — is referenced by this program.
    Dropping the three dead ones lets the barrier (and thus the first
    load's descriptor gen) clear ~300 ns earlier."""
    blk = nc.m.functions[0].blocks[0]
    insts = blk.instructions
    lead = []
    for i in insts:
        tn = type(i).__name__
        if tn == "InstCall":
            continue
        if tn != "InstMemset":
            break
        lead.append(i)
    if len(lead) != 4:
        return
    for i in lead[1:]:
        if i.sync_info and (i.sync_info.on_wait or i.sync_info.on_update):
            return
    for i in lead[1:]:
        insts.remove(i)


def _hoist_first_load(nc):
    """The first load's DMACopy has no waits (fresh SBUF, input valid at
    NEFF start), but it sits after the entry barrier in SP's stream, so
    its descriptor gen can't start until the barrier clears (~330 ns).
    Moving it into the preamble block ahead of SP's barrier instruction
    starts the descgen at t~0 and the first HBM transfer ~380 ns earlier.
    Its completion semaphore fires ~8 us in, long after any preamble sem
    setup, so the reorder cannot race the barrier."""
    blocks = nc.m.functions[0].blocks
    if len(blocks) < 2:
        return
    b0, b1 = blocks[0], blocks[1]
    for i in list(b1.instructions):
        if type(i).__name__ != "InstDMACopy":
            continue
        if not str(i.engine).endswith("SP"):
            continue
        if i.sync_info and i.sync_info.on_wait:
            return
        b1.instructions.remove(i)
        b0.instructions.insert(1, i)
        return


def _run(x_full, trace=False, tmpdir=None):
    """x_full: [1024, 16384] f32. Returns (y_full [1024, 4096] f32, results obj)."""
    if "nc" not in _CACHE:
        _CACHE["nc"] = _build_program()
    nc = _CACHE["nc"]
    in_maps = [
        {"x": np.ascontiguousarray(x_full[c * BSH : (c + 1) * BSH])}
        for c in range(N_CORES)
    ]
    res = run_bass_kernel_spmd(
        nc, in_maps, list(range(N_CORES)), trace=trace, tmpdir=tmpdir
    )
    y_full = np.concatenate([res.results[c]["y"] for c in range(N_CORES)], axis=0)
    return y_full, res


def kernel(input_state, T=None, **_unused):
    x = np.asarray(input_state, dtype=np.float32)
    assert x.shape == (BATCH, IN_F), x.shape
    y, _ = _run(x, trace=False)
    return y
